# revision 16
# baseline (speedup 1.0000x reference)
"""MoE (dense-activated, 32 experts) Trainium2 kernel — v3.

Problem: out[b,t,u] = sum_e gate[b,t,e] * LeakyReLU((x @ We[e] + be[e]))[u]
         gate = x @ Wg + bg   (no softmax)
Shapes: x[32,512,128], Wg[128,32], bg[32], We[32,128,64], be[32,64] -> out[32,512,64]

Strategy: data-parallel over batch across 8 NeuronCores (4 batches = 2048
tokens per core), weights replicated, no collectives. Host pre-transposes
x so the contraction dim F=128 lands on SBUF partitions.

v3 dataflow (v1 in kernel_v1_backup.py):
  - Host column layout [Wgp(64) | x0(512) | We(2048) | x1 | x2 | x3]; one
    leading DMA [0:2624] carries everything tiles 0-3 need.
  - h for a whole tile in ONE 4-bank PSUM tile [128,2048]; ONE Prelu per
    tile (fewer ACT instruction overheads). Gate matmul groups (4 tiles)
    share the same PSUM pool, one ACT copy per group.
  - ONE pair-trick bf16 multiply per tile [128,2048] at DVE 2x_1P.
  - Add-tree lvl0 (1024-wide) on the gpsimd-queue accumulate-DMA
    (SWDGE read-modify-write) - toggle DMA_LVL0; lvl1-3 + fp32 final on DVE.
  - Emission is software-pipelined: the back stage (tree+final+out-DMA) of
    tile i is emitted LAG tiles later, so the in-order DVE stream never
    stalls on the accum-DMA's ~3us latency.

Engine budget per tile (cost model): DVE ~1.7us, ACT ~1.9us, PE ~1.0us,
Pool(SWDGE) ~1.1us -> ~2us/tile steady, DVE/ACT co-bound.

Measured notes: DVE 2x_1P is real on HW (microbench: contig bf16 adds ~4x
faster than fp32); cost model tracks stock-DVE HW costs well; For_i loop
boundary ~1.9us/iter inflates R-slope numbers. GPSIMD compute offload
measured 1.5-2x worse than modeled (prev session) - left out.
"""

import os
import sys

import numpy as np

for _p in ("/opt/trn_rl_repo", os.path.expanduser("~/.axon_site/_ro/trn_rl_repo")):
    if os.path.isdir(_p) and _p not in sys.path:
        sys.path.insert(0, _p)

import concourse.bass as bass  # noqa: F401
import concourse.bacc as bacc
import concourse.tile as tile
from concourse import mybir
from concourse.bass_utils import run_bass_kernel_spmd

ALPHA = 0.01

B, T, F, U, E = 32, 512, 128, 64, 32
N_CORES = 8
TOK = (B // N_CORES) * T          # tokens per core = 2048
P = 128                           # tokens per tile
N_TILES = TOK // P                # 16
EU = E * U                        # 2048
W_COLS = E * U + 2 * E            # 2112 = [Wg-paired | We_flat]
E_HALF = E // 2
HCOLS = E_HALF * U                # 1024

# v3 column layout inside XW [F, 4160]
GOFF = 0                          # paired gate weights (64)
X0OFF = 2 * E                     # x chunk 0 (tiles 0-3)
HOFF = X0OFF + 512                # We_flat (2048)
X1OFF = HOFF + EU                 # x chunks 1-3 (tiles 4-15)
XW_COLS = TOK + W_COLS            # 4160

f32 = mybir.dt.float32
f32r = mybir.dt.float32r
bf16 = mybir.dt.bfloat16

DMA_LVL0 = os.environ.get("DMA_LVL0", "1") == "1"
GATE_GRP = int(os.environ.get("GATE_GRP", "4"))   # tiles per gate group
LAG = int(os.environ.get("LAG", "2"))             # back-stage emission lag
DVE_LVL0_FROM = int(os.environ.get("DVE_LVL0_FROM", "15"))  # tiles >= this do lvl0 on DVE
F32_XW = os.environ.get("F32_XW", "0") == "1"     # fp32 input fallback

_CACHED = {}


def _xcol(i):
    """First XW column of tile i's x slice."""
    if i < 4:
        return X0OFF + i * P
    return X1OFF + (i - 4) * P


def _build_nc(reps=1):
    """Build the single-core SPMD Bass module."""
    from contextlib import ExitStack

    nc = bacc.Bacc("TRN2")
    dt_xw = f32r if F32_XW else bf16
    XW = nc.declare_dram_parameter("XW", [F, XW_COLS], dt_xw, isOutput=False)
    O = nc.declare_dram_parameter("O", [TOK, U], f32, isOutput=True)

    with ExitStack() as ctx:
        tc = ctx.enter_context(tile.TileContext(nc))
        singles = ctx.enter_context(tc.tile_pool(name="singles", bufs=1))
        hlp = ctx.enter_context(tc.tile_pool(name="hlp", bufs=int(os.environ.get("HLP_BUFS", "3"))))
        t1p = ctx.enter_context(tc.tile_pool(name="t1p", bufs=int(os.environ.get("T1P_BUFS", "7" if DMA_LVL0 else "4"))))
        outp = ctx.enter_context(tc.tile_pool(name="outp", bufs=int(os.environ.get("OUTP_BUFS", "2"))))
        gsb = ctx.enter_context(tc.tile_pool(name="gsb", bufs=2))
        ph = ctx.enter_context(tc.tile_pool(name="ph", bufs=int(os.environ.get("PH_BUFS", "2")), space="PSUM"))

        xw = singles.tile([F, XW_COLS], dt_xw)
        # D1 = [Wgp | x0]: the gate matmuls of group 0 start ~2us in and
        # warm the PE p-state; D2 = We; D3-5 = x chunks 1-3.
        nc.sync.dma_start(out=xw[:, 0:HOFF], in_=XW[:, 0:HOFF])
        nc.sync.dma_start(out=xw[:, HOFF:X1OFF], in_=XW[:, HOFF:X1OFF])
        for c in range(3):
            s = X1OFF + c * 512
            nc.sync.dma_start(out=xw[:, s:s + 512], in_=XW[:, s:s + 512])

        # uneven gate groups: {0..3} as soon as x0 lands, {4..15} once the
        # rest of x is in - two PSUM-slot steals and two ACT copies total
        GATE_GROUPS = [(0, 4), (4, 16)] if GATE_GRP == 4 else [
            (g * GATE_GRP, (g + 1) * GATE_GRP)
            for g in range(N_TILES // GATE_GRP)]
        g2_tiles = {}
        o_quads = {}

        def emit_gate_group(g):
            lo, hi = GATE_GROUPS[g]
            g_ps = ph.tile([P, EU], f32, tag="hps")
            for t, i in enumerate(range(lo, hi)):
                nc.tensor.matmul(
                    g_ps[:, t * 2 * E:(t + 1) * 2 * E],
                    lhsT=xw[:, _xcol(i):_xcol(i) + P],
                    rhs=xw[:, GOFF:GOFF + 2 * E],
                    start=True, stop=True,
                )
            g2 = gsb.tile([P, (hi - lo) * 2 * E], bf16)
            nc.scalar.activation(g2[:], g_ps[:, 0:(hi - lo) * 2 * E],
                                 mybir.ActivationFunctionType.Copy)
            for i in range(lo, hi):
                g2_tiles[i] = (g2, (i - lo) * 2 * E)

        def front(i):
            if i == 0:
                emit_gate_group(0)
                if GATE_GROUPS[1][0] < 3:
                    emit_gate_group(1)
            xt_r = xw[:, _xcol(i):_xcol(i) + P]
            g2, goff = g2_tiles[i]

            h_ps = ph.tile([P, EU], f32, tag="hps")
            for j in range(4):  # four 512-col (one-bank) matmuls
                c0 = HOFF + j * 512
                nc.tensor.matmul(
                    h_ps[:, j * 512:(j + 1) * 512],
                    lhsT=xt_r, rhs=xw[:, c0:c0 + 512],
                    start=True, stop=True,
                )
            # ONE LeakyReLU PSUM -> SBUF bf16 for the whole tile
            # (ACT_FUNC_SAFE swaps in Abs for the local interpreter|timing)
            hl = hlp.tile([P, EU], bf16)
            func = (mybir.ActivationFunctionType.Abs
                    if os.environ.get("ACT_FUNC_SAFE", "0") == "1"
                    else mybir.ActivationFunctionType.Prelu)
            nc.scalar.activation(hl[:], h_ps[:], func, alpha=ALPHA)

            # ONE t1 = hl * gate multiply (pair-duplicated gate -> packed
            # (2,1) innermost on every operand -> DVE 2x_1P)
            t1 = t1p.tile([P, EU], bf16)
            hl4 = hl[:].rearrange("p (e u2 two) -> p e u2 two", e=E, two=2)
            g24 = (g2[:, goff:goff + 2 * E]
                   .rearrange("p (e two) -> p e two", two=2)
                   .unsqueeze(2).broadcast_to([P, E, U // 2, 2]))
            t14 = t1[:].rearrange("p (e u2 two) -> p e u2 two", e=E, two=2)
            nc.vector.tensor_tensor(t14, hl4, g24, op=mybir.AluOpType.mult)

            # pre-emit the next gate group so its matmuls hide in PE slack
            # and the ACT copy lands before any Prelu needs to wait on it
            for g, (lo, _hi) in enumerate(GATE_GROUPS):
                if i == lo - 3:
                    emit_gate_group(g)

            if DMA_LVL0 and i < DVE_LVL0_FROM:
                # tree lvl0 as SWDGE accumulate-DMA on the idle Pool queue
                nc.gpsimd.dma_start(
                    out=t1[:, 0:HCOLS], in_=t1[:, HCOLS:EU],
                    accum_op=mybir.AluOpType.add)
                did_dma = True
            else:
                did_dma = False
            return (i, t1, did_dma)

        def back(i, t1, did_dma):
            cur = t1[:]
            width = EU
            for lvl in range(4):
                width //= 2
                nxt = cur[:, 0:width]
                if lvl == 0:
                    if not did_dma:
                        nc.vector.tensor_tensor(
                            nxt, cur[:, 0:width], cur[:, width:2 * width],
                            op=mybir.AluOpType.add)
                    # else: already accumulated by the front-stage DMA
                else:
                    nc.vector.tensor_tensor(
                        nxt, cur[:, 0:width], cur[:, width:2 * width],
                        op=mybir.AluOpType.add)
                cur = nxt
            q, t = divmod(i, 4)
            if t == 0:
                o_quads[q] = outp.tile([P, 4 * U], f32, name="o_q", tag="o_q")
            o_q = o_quads[q]
            # final level as a contiguous fp32-out add (strided reduce_sum
            # is slower on HW than the cost model claims)
            nc.vector.tensor_tensor(
                o_q[:, t * U:(t + 1) * U], cur[:, 0:U], cur[:, U:2 * U],
                op=mybir.AluOpType.add)
            last_q = (q == N_TILES // 4 - 1
                      and os.environ.get("SINGLE_LAST_QUAD", "1") == "1")
            if last_q:
                # ship the final tiles individually: the quad would wait for
                # all four final adds and stretch the drain tail
                nc.sync.dma_start(
                    out=O[i * P:(i + 1) * P, :],
                    in_=o_q[:, t * U:(t + 1) * U])
            elif t == 3:
                # one DMA ships 4 tiles' outputs (saves HWDGE serialization,
                # compresses the drain tail)
                nc.sync.dma_start(
                    out=O[q * 4 * P:(q + 1) * 4 * P, :]
                        .rearrange("(t p) u -> p t u", t=4),
                    in_=o_q[:].rearrange("p (t u) -> p t u", t=4))

        def sweep():
            o_quads.clear()
            pending = []
            for i in range(N_TILES):
                pending.append(front(i))
                # drain the backlog toward the end so the last tiles'
                # tree+output work doesn't all serialize after front(15)
                if os.environ.get("CAP_DRAIN", "1") == "1":
                    cap = LAG if i < N_TILES - 3 else max(N_TILES - 1 - i, 0)
                else:
                    cap = LAG
                while len(pending) > cap:
                    back(*pending.pop(0))
            for st in pending:
                back(*st)

        if reps == 1:
            sweep()
        elif os.environ.get("UNROLL", "1") == "1":
            for _r in range(reps):
                sweep()
        else:
            body = int(os.environ.get("BODY_SWEEPS", "1"))
            with tc.For_i(0, reps, 1):
                for _b in range(body):
                    sweep()

    nc.finalize()
    return nc


def _numpy_fallback(x, Wg, bg, We, be):
    gate = np.einsum("btf,fe->bte", x, Wg) + bg
    h = np.einsum("btf,efu->btue", x, We) + be.T
    h = np.where(h >= 0, h, ALPHA * h)
    return np.einsum("btue,bte->btu", h, gate).astype(np.float32)


LAST_RESULTS = None


def kernel(x, Wg, bg, We, be):
    x = np.asarray(x, dtype=np.float32)
    Wg = np.asarray(Wg, dtype=np.float32)
    bg = np.asarray(bg, dtype=np.float32)
    We = np.asarray(We, dtype=np.float32)
    be = np.asarray(be, dtype=np.float32)

    # device fast path assumes zero biases (true for this problem's inputs)
    if np.any(bg) or np.any(be):
        return _numpy_fallback(x, Wg, bg, We, be)

    if "nc" not in _CACHED:
        _CACHED["nc"] = _build_nc()
    nc = _CACHED["nc"]

    Wgp = np.repeat(Wg, 2, axis=1)                       # [F, 64] paired
    Wef = We.transpose(1, 0, 2).reshape(F, E * U)        # [F, 2048] e-major

    import ml_dtypes

    dt_host = np.float32 if F32_XW else ml_dtypes.bfloat16
    xs = x.reshape(N_CORES, TOK, F)
    in_maps = []
    for c in range(N_CORES):
        xT = xs[c].T                                     # [F, 2048]
        xw = np.concatenate(
            [Wgp, xT[:, 0:512], Wef, xT[:, 512:2048]], axis=1)
        in_maps.append({"XW": np.ascontiguousarray(xw.astype(dt_host))})

    global LAST_RESULTS
    res = run_bass_kernel_spmd(nc, in_maps, list(range(N_CORES)))
    LAST_RESULTS = res
    out = np.stack([res.results[c]["O"] for c in range(N_CORES)], axis=0)
    return out.reshape(B, T, U)
